# revision 1
# baseline (speedup 1.0000x reference)
"""Trainium2 Bass kernel for nn_MILPAttention (dense multi-head attention with
per-key additive bias), tensor-parallel over heads across 8 NeuronCores.

Self-contained: hardcodes shapes N=4096, D=1024, H=16, GAMMA=1.0.

Math (reference):
    q = x @ Wq.T + bq ; k = x @ Wk.T + bk ; v = x @ Wv.T + bv     (per head, dh=64)
    logits = (q @ k.T) / 8 - h[key]
    attn = softmax(logits, keys)
    out = (attn @ v) @ Wo.T + bo + x

Per-core strategy (core i owns heads 2i, 2i+1 = columns 128i:128(i+1)):
    - Projections computed transposed: qT,kT [128, 4096] = W.T.T @ x.T with the
      1/8 scale folded into Wq/bq on the host. v computed transposed then
      PE-transposed to natural [keys, 64] layout and pre-scaled by w=exp(-h)
      (folds the per-key softmax bias into V); w appended as a 65th column so
      the P@V matmul also yields the softmax denominator.
    - S^T[key, q] = kT.T @ qT per head (K=64 contraction), exp on ScalarE
      (no max subtraction: logits are bounded ~ +-12), P kept bf16.
    - outT[d, q] (+denominator row) = vw.T @ P^T accumulated over key chunks.
    - Normalize by broadcasting 1/denominator, AllToAll to switch from
      head-sharding to sequence-sharding, then the output projection + bias
      + residual for this core's 512 rows.
"""
import numpy as np

import concourse.bass as bass
import concourse.mybir as mybir
import concourse.tile as tile
from concourse import bacc
from concourse.bass_utils import run_bass_kernel_spmd
from concourse.masks import make_identity

N, D, H = 4096, 1024, 16
NCORE = 8
CB = D // NCORE          # 128 columns (2 heads) per core
NR = N // NCORE          # 512 output rows per core
DH = D // H              # 64
KCH = N // 128           # 32 key chunks
NB = N // 512            # 8 n-blocks
BQ = 1024                # per-head q-block width in attention phase
QB = N // BQ             # 4 q-blocks
FP = mybir.dt.float32
BF = mybir.dt.bfloat16
AF = mybir.ActivationFunctionType


def _body(nc, tc, reps, xt, xr, wqt, wkt, wvt, wot, bqv, bkv, bvv, bov, hv, out,
          dbg=None, use_collective=True):
    cst = tc.alloc_tile_pool(name="cst", bufs=1)
    per = tc.alloc_tile_pool(name="per", bufs=1)
    dram = tc.alloc_tile_pool(name="dram", bufs=1, space="DRAM")

    ident = cst.tile([128, 128], BF)
    make_identity(nc, ident[:])

    # persistent sbuf
    wq_b = per.tile([128, D], BF)        # [d-in-chunk, dc*128 + c]
    wk_b = per.tile([128, D], BF)
    wv_b = per.tile([128, D], BF)
    wo_b = per.tile([128, 8 * D], BF)    # [c-in-chunk, cc*1024 + o]
    qb_t = per.tile([128, N], BF)        # qT: rows = 2 heads x 64 dims
    kb_t = per.tile([128, N], BF)
    vw_a = per.tile([128, KCH * 130], BF)  # per key chunk: 65 cols per head
    ao_s = per.tile([128, N], BF)        # normalized attn-out^T
    bq_s = per.tile([128, 1], FP)
    bk_s = per.tile([128, 1], FP)
    bv_s = per.tile([128, 1], FP)
    w_s = per.tile([128, KCH], FP)       # exp(-h), [key-in-chunk, chunk]
    xb_s = [per.tile([128, D], FP, name=f"xb{j}") for j in range(4)]  # x rows + bo

    cc_in = dram.tile([NCORE * 128, NR], BF)
    cc_out = dram.tile([NCORE * 128, NR], BF)

    for rep in range(reps):
        sfx = f"_{rep}"
        # ---------------- phase 0: constants (no pool: all tiles persistent,
        # all DMAs on the fast HWDGE queues so nothing gates the xb stream) ---
        hst = per.tile([128, KCH], FP, name="hst")
        nc.sync.dma_start(hst[:], hv)
        nc.scalar.activation(w_s[:], hst[:], AF.Exp, scale=-1.0)
        nc.sync.dma_start(bq_s[:], bqv.unsqueeze(1))
        nc.scalar.dma_start(bk_s[:], bkv.unsqueeze(1))
        nc.scalar.dma_start(bv_s[:], bvv.unsqueeze(1))
        for wi, (wsrc, wdst) in enumerate(
                ((wqt, wq_b), (wkt, wk_b), (wvt, wv_b))):
            eng = (nc.sync, nc.scalar, nc.scalar)[wi]
            # [D, CB] bf16 -> [128, dc*128 + c] in one rearranged DMA
            eng.dma_start(wdst[:].rearrange("p (dc c) -> p dc c", c=CB),
                          wsrc.rearrange("(dc p) c -> p dc c", p=128))

        # ------- phases 1+2 fully woven ------------------------------------
        # All projections are emitted as blocks INSIDE the attention stream:
        # k/v blocks pipelined a few key-chunks ahead of their first use in
        # (qb0, h0); q blocks woven into (qb0, h1). Engines execute their
        # streams in program order, so emission order controls the overlap —
        # the ScalarE exp stream starts almost immediately and stays hot.
        # PSUM budget: pss 2x2 + pso 1x2 + shared proj bank + pvt bank = 8.
        with tc.tile_pool(name="p1s" + sfx, bufs=6) as p1s, \
             tc.tile_pool(name="p1p", bufs=1, space="PSUM") as p1p, \
             tc.tile_pool(name="p2s" + sfx, bufs=3) as p2s, \
             tc.tile_pool(name="p2n", bufs=2) as p2n, \
             tc.tile_pool(name="p2ps", bufs=2, space="PSUM") as p2ps, \
             tc.tile_pool(name="p2po", bufs=1, space="PSUM") as p2po:
            def kv_block(nb):
                xtb = []
                for dc in range(8):
                    xb = p1s.tile([128, 512], BF, name="xb")
                    (nc.sync if dc % 2 == 0 else nc.scalar).dma_start(
                        xb[:], xt[dc * 128:(dc + 1) * 128, nb * 512:(nb + 1) * 512])
                    xtb.append(xb)
                psk = p1p.tile([128, 512], FP, name="proj")
                for dc in range(8):
                    nc.tensor.matmul(psk[:], wk_b[:, dc * CB:(dc + 1) * CB], xtb[dc][:],
                                     start=(dc == 0), stop=(dc == 7))
                ncol = slice(nb * 512, (nb + 1) * 512)
                nc.vector.tensor_scalar_add(kb_t[:, ncol], psk[:], bk_s[:, 0:1])
                psv = p1p.tile([128, 512], FP, name="pvt")
                for dc in range(8):
                    nc.tensor.matmul(psv[:], wv_b[:, dc * CB:(dc + 1) * CB], xtb[dc][:],
                                     start=(dc == 0), stop=(dc == 7))
                vtb = p1s.tile([128, 512], BF, name="vtb")
                nc.vector.tensor_scalar_add(vtb[:], psv[:], bv_s[:, 0:1])
                for ns in range(4):
                    kc = nb * 4 + ns
                    pvt = p1p.tile([128, 128], BF, name="pvt")
                    nc.tensor.transpose(pvt[:], vtb[:, ns * 128:(ns + 1) * 128], ident[:])
                    c0 = kc * 130
                    nc.vector.tensor_scalar_mul(
                        vw_a[:, c0:c0 + 64], pvt[:, 0:64], w_s[:, kc:kc + 1])
                    nc.vector.tensor_copy(vw_a[:, c0 + 64:c0 + 65], w_s[:, kc:kc + 1])
                    nc.vector.tensor_scalar_mul(
                        vw_a[:, c0 + 65:c0 + 129], pvt[:, 64:128], w_s[:, kc:kc + 1])
                    nc.vector.tensor_copy(vw_a[:, c0 + 129:c0 + 130], w_s[:, kc:kc + 1])

            def qproj_block(nb):
                xtb = []
                for dc in range(8):
                    xb2 = p1s.tile([128, 512], BF, name="xb2")
                    (nc.sync if dc % 2 == 0 else nc.scalar).dma_start(
                        xb2[:], xt[dc * 128:(dc + 1) * 128, nb * 512:(nb + 1) * 512])
                    xtb.append(xb2)
                psq = p1p.tile([128, 512], FP, name="proj")
                for dc in range(8):
                    nc.tensor.matmul(psq[:], wq_b[:, dc * CB:(dc + 1) * CB], xtb[dc][:],
                                     start=(dc == 0), stop=(dc == 7))
                nc.vector.tensor_scalar_add(qb_t[:, nb * 512:(nb + 1) * 512],
                                            psq[:], bq_s[:, 0:1])

            kv_block(0)
            qproj_block(0)
            qproj_block(1)
            kv_next, qp_next = 1, 2

            # q-blocks of 512 with BOTH heads per step: the two K=64 S-matmuls
            # sit in disjoint PE row groups (partitions 0-63 / 64-127) and run
            # concurrently; exp still covers [128, 1024] per call.
            for q5 in range(N // 512):
                qcol = slice(q5 * 512, (q5 + 1) * 512)
                pso = [p2po.tile([65, 512], FP, name=f"pso{h}") for h in range(2)]
                for kc in range(KCH):
                    krng = slice(kc * 128, (kc + 1) * 128)
                    pss = p2ps.tile([128, 1024], FP, name="pss")
                    for h in range(2):
                        hr = slice(h * 64, (h + 1) * 64)
                        nc.tensor.matmul(pss[:, h * 512:(h + 1) * 512],
                                         kb_t[hr, krng], qb_t[hr, qcol],
                                         start=True, stop=True)
                    pb = p2s.tile([128, 1024], BF, name="pb")
                    nc.scalar.activation(pb[:], pss[:], AF.Exp)
                    for h in range(2):
                        lh = vw_a[:, kc * 130 + 65 * h: kc * 130 + 65 * h + 65]
                        nc.tensor.matmul(
                            pso[h][:], lh, pb[:, h * 512:(h + 1) * 512],
                            start=(kc == 0), stop=(kc == KCH - 1))
                    # weave projection blocks under the exp stream
                    if q5 == 0 and kc % 4 == 0 and kv_next < NB:
                        kv_block(kv_next)
                        kv_next += 1
                    if q5 in (1, 2) and kc % 6 == 5 and qp_next < NB:
                        qproj_block(qp_next)
                        qp_next += 1
                # snap pso to SBUF fast (pso tiles are single-buffered), then
                # normalize from the copy: ao = snap[0:64]/snap[64]
                for h in range(2):
                    snap = p2n.tile([65, 512], FP, name=f"snap{h}")
                    nc.vector.tensor_copy(snap[:], pso[h][:])
                    rc = p2n.tile([1, 512], FP, name=f"rc{h}")
                    nc.vector.reciprocal(rc[:], snap[64:65, :])
                    bch = p2n.tile([64, 512], FP, name=f"bc{h}")
                    nc.gpsimd.partition_broadcast(bch[:], rc[:])
                    nc.vector.tensor_mul(ao_s[h * 64:(h + 1) * 64, qcol],
                                         snap[0:64, :], bch[:])

        # prefetch phase-3 constants: queued behind the projection DMAs so the
        # transfers land during phase 2 (DMA is idle there)
        with tc.tile_pool(name="pf" + sfx, bufs=2) as pf:
            nc.sync.dma_start(wo_b[:].rearrange("p (cc o) -> p cc o", o=D),
                              wot.rearrange("(cc p) o -> p cc o", p=128))
            bost = pf.tile([128, D], FP, name="bost")
            nc.sync.dma_start(bost[:], bov.unsqueeze(0).broadcast_to([128, D]))
            for j in range(4):
                xrt = pf.tile([128, D], FP, name="xrt")
                (nc.sync if j % 2 == 0 else nc.scalar).dma_start(
                    xrt[:], xr[j * 128:(j + 1) * 128, :])
                nc.vector.tensor_add(xb_s[j][:], xrt[:], bost[:])

        # ---------------- phase 3: A2A + out projection ----------------
        with tc.tile_pool(name="p3s" + sfx, bufs=2) as p3s, \
             tc.tile_pool(name="p3p", bufs=1, space="PSUM") as p3p:
            for j in range(NCORE):
                nc.sync.dma_start(cc_in[j * 128:(j + 1) * 128, :],
                                  ao_s[:, j * NR:(j + 1) * NR])
            if use_collective:
                nc.gpsimd.collective_compute(
                    "AllToAll", mybir.AluOpType.bypass,
                    replica_groups=[list(range(NCORE))],
                    ins=[cc_in[:].opt()], outs=[cc_out[:].opt()])
            else:  # single-core timing-sim stand-in
                nc.sync.dma_start(cc_out[:], cc_in[:])
            psf = [p3p.tile([128, 512], FP, name=f"psf{t}") for t in range(8)]
            for cc in range(8):
                aoc = p3s.tile([128, NR], BF, name="aoc")
                nc.sync.dma_start(aoc[:], cc_out[cc * 128:(cc + 1) * 128, :])
                if dbg is not None:
                    nc.sync.dma_start(dbg["d_aoc"][cc * 128:(cc + 1) * 128, :], aoc[:])
                for ns in range(4):
                    for ob in range(2):
                        nc.tensor.matmul(
                            psf[ns * 2 + ob][:],
                            aoc[:, ns * 128:(ns + 1) * 128],
                            wo_b[:, cc * D + ob * 512: cc * D + (ob + 1) * 512],
                            start=(cc == 0), stop=(cc == 7))
            for ns in range(4):
                for ob in range(2):
                    fo = p3s.tile([128, 512], FP, name="fo")
                    nc.vector.tensor_add(fo[:], psf[ns * 2 + ob][:],
                                         xb_s[ns][:, ob * 512:(ob + 1) * 512])
                    nc.sync.dma_start(
                        out[ns * 128:(ns + 1) * 128, ob * 512:(ob + 1) * 512], fo[:])

    if dbg is not None:
        for nm, t in (("d_q", qb_t), ("d_k", kb_t), ("d_vw", vw_a), ("d_ao", ao_s)):
            nc.sync.dma_start(dbg[nm], t[:])
        nc.sync.dma_start(dbg["d_cc"], cc_out[:])

    dram.release()
    per.release()
    cst.release()


def build_nc(reps=1, debug=False, use_collective=True):
    nc = bacc.Bacc("TRN2", target_bir_lowering=False, debug=False, num_devices=NCORE)
    xt = nc.dram_tensor("xt", [D, N], BF, kind="ExternalInput").ap()
    xr = nc.dram_tensor("xr", [NR, D], FP, kind="ExternalInput").ap()
    wqt = nc.dram_tensor("wqt", [D, CB], BF, kind="ExternalInput").ap()
    wkt = nc.dram_tensor("wkt", [D, CB], BF, kind="ExternalInput").ap()
    wvt = nc.dram_tensor("wvt", [D, CB], BF, kind="ExternalInput").ap()
    wot = nc.dram_tensor("wot", [D, D], BF, kind="ExternalInput").ap()
    bqv = nc.dram_tensor("bqv", [CB], FP, kind="ExternalInput").ap()
    bkv = nc.dram_tensor("bkv", [CB], FP, kind="ExternalInput").ap()
    bvv = nc.dram_tensor("bvv", [CB], FP, kind="ExternalInput").ap()
    bov = nc.dram_tensor("bov", [D], FP, kind="ExternalInput").ap()
    # h pre-rearranged on host to [128, KCH]: hv[p, c] = h[c*128 + p]
    hv = nc.dram_tensor("hv", [128, KCH], FP, kind="ExternalInput").ap()
    out = nc.dram_tensor("out", [NR, D], FP, kind="ExternalOutput").ap()
    dbg = None
    if debug:
        dbg = {
            "d_q": nc.dram_tensor("d_q", [128, N], BF, kind="ExternalOutput").ap(),
            "d_k": nc.dram_tensor("d_k", [128, N], BF, kind="ExternalOutput").ap(),
            "d_vw": nc.dram_tensor("d_vw", [128, KCH * 130], BF, kind="ExternalOutput").ap(),
            "d_ao": nc.dram_tensor("d_ao", [128, N], BF, kind="ExternalOutput").ap(),
            "d_cc": nc.dram_tensor("d_cc", [NCORE * 128, NR], BF, kind="ExternalOutput").ap(),
            "d_aoc": nc.dram_tensor("d_aoc", [NCORE * 128, NR], BF, kind="ExternalOutput").ap(),
        }
    with tile.TileContext(nc) as tc:
        _body(nc, tc, reps, xt, xr, wqt, wkt, wvt, wot,
              bqv, bkv, bvv, bov, hv, out, dbg=dbg, use_collective=use_collective)
    nc.compile()
    return nc


_NC_CACHE = {}


def get_nc(reps=1):
    if reps not in _NC_CACHE:
        _NC_CACHE[reps] = build_nc(reps)
    return _NC_CACHE[reps]


def make_in_maps(inputs):
    x = np.ascontiguousarray(np.asarray(inputs["x"], dtype=np.float32))
    h = np.ascontiguousarray(np.asarray(inputs["h"], dtype=np.float32))
    Wq = np.asarray(inputs["Wq"], dtype=np.float32)
    bq = np.asarray(inputs["bq"], dtype=np.float32)
    Wk = np.asarray(inputs["Wk"], dtype=np.float32)
    bk = np.asarray(inputs["bk"], dtype=np.float32)
    Wv = np.asarray(inputs["Wv"], dtype=np.float32)
    bv = np.asarray(inputs["bv"], dtype=np.float32)
    Wo = np.asarray(inputs["Wo"], dtype=np.float32)
    bo = np.ascontiguousarray(np.asarray(inputs["bo"], dtype=np.float32))
    import ml_dtypes
    bf16 = ml_dtypes.bfloat16
    xt = np.ascontiguousarray(x.T.astype(bf16))
    wot = np.ascontiguousarray(Wo.T.astype(bf16))
    scale = np.float32(0.125)  # 1/sqrt(dh), folded into q
    in_maps = []
    for i in range(NCORE):
        cs = slice(i * CB, (i + 1) * CB)
        in_maps.append({
            "xt": xt,
            "xr": np.ascontiguousarray(x[i * NR:(i + 1) * NR, :]),
            "wqt": np.ascontiguousarray((Wq[cs, :] * scale).T.astype(bf16)),
            "wkt": np.ascontiguousarray(Wk[cs, :].T.astype(bf16)),
            "wvt": np.ascontiguousarray(Wv[cs, :].T.astype(bf16)),
            "wot": wot,
            "bqv": np.ascontiguousarray(bq[cs] * scale),
            "bkv": np.ascontiguousarray(bk[cs]),
            "bvv": np.ascontiguousarray(bv[cs]),
            "bov": bo,
            "hv": np.ascontiguousarray(h.reshape(KCH, 128).T),
        })
    return in_maps


def kernel(**inputs):
    nc = get_nc(1)
    in_maps = make_in_maps(inputs)
    res = run_bass_kernel_spmd(nc, in_maps, core_ids=list(range(NCORE)))
    return np.concatenate([res.results[i]["out"] for i in range(NCORE)], axis=0)





# revision 36
# speedup vs baseline: 6.7006x; 6.7006x over previous
"""Trainium2 Bass kernel for nn_MILPAttention (dense multi-head attention with
per-key additive bias), tensor-parallel over heads across 8 NeuronCores.

Self-contained: hardcodes shapes N=4096, D=1024, H=16, GAMMA=1.0.

Math (reference):
    q = x @ Wq.T + bq ; k = x @ Wk.T + bk ; v = x @ Wv.T + bv     (per head, dh=64)
    logits = (q @ k.T) / 8 - h[key]
    attn = softmax(logits, keys)
    out = (attn @ v) @ Wo.T + bo + x

Per-core strategy (core i owns heads 2i, 2i+1 = columns 128i:128(i+1)):
  - Projections transposed: kT [128, 4096] = Wk.T.T @ x.T with x resident in
    SBUF (loaded once). q is computed into TWO half-zeroed tiles qA (head0 in
    rows 0:64, rows 64:128 = 0) and qB (head1 in rows 64:128, rows 0:64 = 0)
    so every S matmul runs with full K=128 contraction (K=64 matmuls run at
    half rate on TRN2). q is pre-scaled by m8 = 8/ln2 (Schraudolph prep).
  - S^T[key, q] in PSUM = m8 * logits. P' = exp(l - c) is produced in
    fp8e4m3, split between ScalarE (true exp via activation scale/bias) and
    VectorE (Schraudolph: int8 convert of m8*l + B' with saturating max,
    bits reinterpreted as fp8e4m3). The global shift c keeps P' in fp8
    range; it cancels in the softmax ratio.
  - P@V runs as fp8 DoubleRow matmuls (two key chunks per pass, 2x PE
    throughput). V is transposed via the PE, scaled by w = exp(-h) (folds
    the per-key bias), stored fp8 with w appended as a 65th column so the
    matmul also yields the softmax denominator.
  - Normalize uses reciprocal_approx_fast on the denominator rows, gpsimd
    partition-broadcast, and a vector multiply writing fp8. AllToAll (fp8)
    switches from head-sharding to sequence-sharding; the out projection
    runs as fp8 DoubleRow over chunk pairs, then bias + residual in fp32.
"""
import numpy as np

import concourse.bass as bass
import concourse.mybir as mybir
import concourse.tile as tile
from concourse import bacc
from concourse.bass_utils import run_bass_kernel_spmd
from concourse.masks import make_identity

N, D, H = 4096, 1024, 16
NCORE = 8
CB = D // NCORE          # 128 columns (2 heads) per core
NR = N // NCORE          # 512 output rows per core
DH = D // H              # 64
KCH = N // 128           # 32 key chunks
NB = N // 512            # 8 n-blocks
FP = mybir.dt.float32
BF = mybir.dt.bfloat16
F8 = mybir.dt.float8e4
I8 = mybir.dt.int8
AF = mybir.ActivationFunctionType
ALU = mybir.AluOpType
DR = mybir.MatmulPerfMode.DoubleRow

M8 = 8.0 / np.log(2.0)      # 11.5416 Schraudolph scale, folded into Wq on host
CSHIFT = 4.8                # global logit shift: P' = exp(l - c), cancels in softmax
                            # (fp8e4m3 here is IEEE e4m3: max normal 240, inf above)
CORR = -0.47                # Schraudolph correction (round-to-nearest tuned)
BPRIME = 56.0 + CORR - M8 * CSHIFT   # DVE add constant


def _body(nc, tc, reps, xt, xr, wqt, wkt, wvt, wot, bqv, bkv, bvv, bov, hv, out,
          use_collective=True, dbg=None):
    cst = tc.alloc_tile_pool(name="cst", bufs=1)
    per = tc.alloc_tile_pool(name="per", bufs=1)
    dram = tc.alloc_tile_pool(name="dram", bufs=1, space="DRAM")

    ident = cst.tile([128, 128], BF)
    make_identity(nc, ident[:])

    # persistent sbuf
    wq_b = per.tile([128, D], F8)        # [d-in-chunk, dc*128 + c]
    wk_b = per.tile([128, D], F8)
    wv_b = per.tile([128, D], F8)
    wo_b = per.tile([128, 8 * D], F8)    # [c-in-chunk, cc*1024 + o]
    qa_t = per.tile([128, N], BF)        # head0 q in rows 0:64, zeros 64:128
    qb_t = per.tile([128, N], BF)        # head1 q in rows 64:128, zeros 0:64
    kb_t = per.tile([128, N], BF)        # kT: rows = 2 heads x 64 dims
    vw8 = per.tile([128, KCH * 256], F8)  # per kc, per head: [w|pad63|v64]
    bq_s = per.tile([128, 1], FP)
    bk_s = per.tile([128, 1], FP)
    bv_s = per.tile([128, 1], FP)
    w_s = per.tile([128, KCH], FP)       # exp(-h), [key-in-chunk, chunk]
    cbias = per.tile([128, 1], FP)       # -CSHIFT for the exp activation
    cscale = per.tile([128, 1], FP)      # 1/M8 for the exp activation
    ones_r = per.tile([1, 128], FP)      # K=1 stationary for denom fan-out
    xb_s = [per.tile([128, D], FP, name=f"xb{j}") for j in range(4)]  # x rows + bo
    x_all = per.tile([128, 8 * N], F8)   # x^T resident, col = dc*N + n

    cc_in = [dram.tile([NCORE * 128, NR // 2], F8, name=f"ccin{i}") for i in range(2)]
    cc_out = [dram.tile([NCORE * 128, NR // 2], F8, name=f"ccout{i}") for i in range(2)]

    for rep in range(reps):
        sfx = f"_{rep}"
        # ---------------- phase 0: constants ------------------------------
        hst = per.tile([128, KCH], FP, name="hst")
        nc.sync.dma_start(hst[:], hv)
        nc.scalar.activation(w_s[:], hst[:], AF.Exp, scale=-1.0)
        nc.sync.dma_start(bq_s[:], bqv.unsqueeze(1))
        nc.scalar.dma_start(bk_s[:], bkv.unsqueeze(1))
        nc.scalar.dma_start(bv_s[:], bvv.unsqueeze(1))
        for wi, (wsrc, wdst) in enumerate(
                ((wqt, wq_b), (wkt, wk_b), (wvt, wv_b))):
            eng = (nc.sync, nc.scalar, nc.scalar)[wi]
            # [D, CB] bf16 -> [128, dc*128 + c] in one rearranged DMA
            eng.dma_start(wdst[:].rearrange("p (dc c) -> p dc c", c=CB),
                          wsrc.rearrange("(dc p) c -> p dc c", p=128))
        # zero halves of qA/qB (once per rep is harmless; writes are cheap)
        nc.vector.memset(qa_t[64:128, :], 0.0)
        nc.gpsimd.memset(qb_t[0:64, :], 0.0)
        nc.gpsimd.memset(cbias[:], -CSHIFT)
        nc.gpsimd.memset(cscale[:], 1.0 / M8)
        nc.gpsimd.memset(ones_r[:], 1.0)
        # w columns of vw8 (first col of each head block) <- w_s[:, kc]
        for h in range(2):
            nc.vector.tensor_copy(
                vw8[:].rearrange("p (kc c) -> p kc c", c=256)[:, :, 128 * h],
                w_s[:])

        # ------- phases 1+2 fully woven ------------------------------------
        # Projection blocks are emitted INSIDE the attention stream: k/v
        # blocks pipelined ahead of their first use, q blocks woven into
        # early q5 iterations. x^T is DMA'd once into resident SBUF tiles.
        with tc.tile_pool(name="p1s" + sfx, bufs=4) as p1s, \
             tc.tile_pool(name="aop" + sfx, bufs=4) as aop, \
             tc.tile_pool(name="p2s" + sfx, bufs=4) as p2s, \
             tc.tile_pool(name="p2n", bufs=2) as p2n, \
             tc.tile_pool(name="p2ps", bufs=2, space="PSUM") as p2ps, \
             tc.tile_pool(name="p2pb", bufs=2, space="PSUM") as p2pb, \
             tc.tile_pool(name="p2po", bufs=1, space="PSUM") as p2po:
            p1p = p2ps  # proj weave shares the psa buffers (name 'pss')
            def load_x(nb):
                for dc in range(8):
                    (nc.sync if dc % 2 == 0 else nc.gpsimd).dma_start(
                        x_all[:, dc * N + nb * 512:dc * N + (nb + 1) * 512],
                        xt[dc * 128:(dc + 1) * 128, nb * 512:(nb + 1) * 512])
                return None

            def kv_block(nb):
                ncol = slice(nb * 512, (nb + 1) * 512)
                psk = p1p.tile([128, 512], FP, name="pss")
                for dp in range(4):
                    nc.tensor.matmul(
                        psk[:],
                        wk_b[:].rearrange("p (dp two c) -> p dp two c",
                                          two=2, c=CB)[:, dp],
                        x_all[:].rearrange("p (dp two n) -> p dp two n",
                                           two=2, n=N)[:, dp, :, ncol],
                        start=(dp == 0), stop=(dp == 3), perf_mode=DR)
                nc.scalar.activation(kb_t[:, ncol], psk[:], AF.Identity, bias=bk_s[:, 0:1])
                psv = p1p.tile([128, 512], FP, name="pss")
                for dp in range(4):
                    nc.tensor.matmul(
                        psv[:],
                        wv_b[:].rearrange("p (dp two c) -> p dp two c",
                                          two=2, c=CB)[:, dp],
                        x_all[:].rearrange("p (dp two n) -> p dp two n",
                                           two=2, n=N)[:, dp, :, ncol],
                        start=(dp == 0), stop=(dp == 3), perf_mode=DR)
                vtb = p1s.tile([128, 512], BF, name="vtb")
                nc.scalar.activation(vtb[:], psv[:], AF.Identity, bias=bv_s[:, 0:1])
                for ns in range(4):
                    kc = nb * 4 + ns
                    pvt = p1p.tile([128, 128], BF, name="pss")
                    nc.tensor.transpose(pvt[:], vtb[:, ns * 128:(ns + 1) * 128], ident[:])
                    c0 = kc * 256
                    nc.vector.tensor_scalar_mul(
                        vw8[:, c0 + 64:c0 + 128], pvt[:, 0:64], w_s[:, kc:kc + 1])
                    nc.vector.tensor_scalar_mul(
                        vw8[:, c0 + 192:c0 + 256], pvt[:, 64:128], w_s[:, kc:kc + 1])

            def qproj_block(nb):
                ncol = slice(nb * 512, (nb + 1) * 512)
                psq = p1p.tile([128, 512], FP, name="pss")
                for dp in range(4):
                    nc.tensor.matmul(
                        psq[:],
                        wq_b[:].rearrange("p (dp two c) -> p dp two c",
                                          two=2, c=CB)[:, dp],
                        x_all[:].rearrange("p (dp two n) -> p dp two n",
                                           two=2, n=N)[:, dp, :, ncol],
                        start=(dp == 0), stop=(dp == 3), perf_mode=DR)
                qa_dst = qa_t[0:64, :].rearrange(
                    "p (s j t) -> p s j t", j=8, t=64)[:, :, nb, :]
                qb_dst = qb_t[64:128, :].rearrange(
                    "p (s j t) -> p s j t", j=8, t=64)[:, :, nb, :]
                nc.vector.tensor_scalar_add(
                    qa_dst, psq[0:64, :].rearrange("p (s t) -> p s t", t=64),
                    bq_s[0:64, 0:1])
                nc.vector.tensor_scalar_add(
                    qb_dst, psq[64:128, :].rearrange("p (s t) -> p s t", t=64),
                    bq_s[64:128, 0:1])

            # striped q layout: every phys q-block needs EVERY qproj block,
            # so x is fully preloaded and all q projections run up front.
            for nb in range(NB):
                load_x(nb)
            kv_block(0)
            for nb in range(NB):
                qproj_block(nb)
            kv_next = 1

            # vw8 stationary AP for (pair p, head h): [128, 2@130, 65]
            vw8_r = vw8[:].rearrange("p (pr two c) -> p pr two c", two=2, c=256)

            for q5 in range(N // 512):
                qcol = slice(q5 * 512, (q5 + 1) * 512)
                pso = [p2po.tile([128, 512], FP, name=f"pso{h}") for h in range(2)]
                pb_q = []  # deferred PV inputs: (pr, pb8_r)

                def emit_pv():
                    pr_, pbr_ = pb_q.pop(0)
                    for h in range(2):
                        nc.tensor.matmul(
                            pso[h][:], vw8_r[:, pr_, :, 128 * h:128 * h + 128],
                            pbr_[:, :, h * 512:(h + 1) * 512],
                            start=(pr_ == 0), stop=(pr_ == 15), perf_mode=DR)

                for pr in range(16):
                    pb8 = p2s.tile([128, 2048], F8, name="pb8")
                    pb8_r = pb8[:].rearrange("p (two c) -> p two c", two=2)
                    psb = p2pb.tile([128, 1024], FP, name="psb")
                    for i in range(2):
                        kc = 2 * pr + i
                        krng = slice(kc * 128, (kc + 1) * 128)
                        # head0 on ScalarE (true exp), head1 on VectorE
                        # (Schraudolph) — any per-engine bias is uniform
                        # within a head and cancels in the softmax ratio.
                        psa = p2ps.tile([128, 512], FP, name="pss")
                        nc.tensor.matmul(psa[:], kb_t[:, krng],
                                         qa_t[:, qcol], start=True, stop=True)
                        nc.scalar.activation(
                            pb8[:, i * 1024:i * 1024 + 512], psa[:],
                            AF.Exp, bias=cbias[:], scale=cscale[:])
                        nc.tensor.matmul(psb[:, i * 512:(i + 1) * 512],
                                         kb_t[:, krng],
                                         qb_t[:, qcol], start=True, stop=True)
                    # one paired Schraudolph op for both head1 halves
                    nc.vector.tensor_scalar(
                        pb8[:].bitcast(I8).rearrange(
                            "p (i c) -> p i c", c=1024)[:, :, 512:1024],
                        psb[:], BPRIME, 0.0, op0=ALU.add, op1=ALU.max)
                    pb_q.append((pr, pb8_r))
                    if pr > 0:
                        emit_pv()  # pair pr-1: its exps finished during this pair
                    # weave k/v projection blocks under the attention stream
                    if q5 == 0 and pr % 2 == 0 and kv_next < NB:
                        kv_block(kv_next)
                        kv_next += 1
                emit_pv()  # last pair
                if q5 == 4 and use_collective:
                    # first-half A2A runs under the remaining attention
                    nc.gpsimd.collective_compute(
                        "AllToAll", mybir.AluOpType.bypass,
                        replica_groups=[list(range(NCORE))],
                        ins=[cc_in[0][:].opt()], outs=[cc_out[0][:].opt()])
                # normalize: ao = snap[0:64] * recip(snap[64]) -> fp8
                # (all DVE ops keep matching partition offsets; row placement
                # into the A2A buffer is done by the staging DMAs)
                for h in range(2):
                    snap = p2n.tile([128, 512], FP, name=f"snap{h}")
                    nc.scalar.copy(snap[:], pso[h][:])
                    rc = p2n.tile([1, 512], FP, name=f"rc{h}")
                    nc.vector.reciprocal_approx_fast(rc[:], snap[0:1, :])
                    if dbg is not None and q5 == 0:
                        nc.sync.dma_start(dbg[f"d_sn{h}"], snap[:])
                    bch = p2n.tile([128, 512], FP, name=f"bc{h}")
                    nc.gpsimd.partition_broadcast(bch[:], rc[:])
                    # rotating output tile: avoids a tile-level WAR between
                    # this write and the previous block's staging-DMA read
                    aot = aop.tile([128, 512], F8, name=f"ao{h}")
                    # mul on VectorE: keeps the gpsimd queue free to issue the
                    # next broadcast immediately, overlapping the ~15us
                    # completion-semaphore latency of each broadcast
                    nc.vector.tensor_mul(aot[64:128, :], snap[64:128, :],
                                         bch[64:128, :])
                    # stage phys block q5 (stripe q5 of every dest) for the A2A
                    half, bc = q5 // 4, q5 % 4
                    nc.sync.dma_start(
                        cc_in[half][:].rearrange("(j g) c -> g j c", g=128)
                        [h * 64:(h + 1) * 64, :, bc * 64:(bc + 1) * 64],
                        aot[64:128, :].rearrange("p (j t) -> p j t", t=64))
                    if dbg is not None:
                        nc.scalar.dma_start(
                            dbg["d_ao"][h * 64:(h + 1) * 64, qcol],
                            aot[64:128, :])

        # prefetch phase-3 constants during phase 2 (DMA idle there)
        with tc.tile_pool(name="pf" + sfx, bufs=2) as pf:
            nc.sync.dma_start(wo_b[:].rearrange("p (cc o) -> p cc o", o=D),
                              wot.rearrange("(cc p) o -> p cc o", p=128))
            bost = pf.tile([128, D], FP, name="bost")
            nc.sync.dma_start(bost[:], bov.unsqueeze(0).broadcast_to([128, D]))
            for j in range(4):
                xrt = pf.tile([128, D], FP, name="xrt")
                (nc.sync if j % 2 == 0 else nc.scalar).dma_start(
                    xrt[:], xr[j * 128:(j + 1) * 128, :])
                nc.vector.tensor_add(xb_s[j][:], xrt[:], bost[:])

        # ---------------- phase 3: A2A + out projection (fp8 DoubleRow) ----
        with tc.tile_pool(name="p3s" + sfx, bufs=8) as p3s, \
             tc.tile_pool(name="p3p", bufs=1, space="PSUM") as p3p:
            if use_collective:
                nc.gpsimd.collective_compute(
                    "AllToAll", mybir.AluOpType.bypass,
                    replica_groups=[list(range(NCORE))],
                    ins=[cc_in[1][:].opt()], outs=[cc_out[1][:].opt()])
            else:  # single-core timing-sim stand-in
                for i in range(2):
                    nc.sync.dma_start(cc_out[i][:], cc_in[i][:])
            psf = [p3p.tile([128, 512], FP, name=f"psf{t}") for t in range(8)]
            wo_r = wo_b[:].rearrange("p (pr two o) -> p pr two o", two=2, o=D)
            for half in range(2):
                # issue ALL aoc loads for this half up front (bufs=8): their
                # ~15us DMA completion semaphores overlap instead of chaining
                aocs = []
                for cp in range(4):
                    aoc = p3s.tile([128, 512], F8, name="aoc")
                    (nc.sync, nc.scalar, nc.gpsimd, nc.sync)[cp].dma_start(
                        aoc[:].rearrange("p (two q) -> p two q", two=2),
                        cc_out[half][cp * 256:(cp + 1) * 256, :].rearrange(
                            "(two p) q -> p two q", p=128))
                    aocs.append(aoc)
                for cp in range(4):
                    aoc_r = aocs[cp][:].rearrange("p (two q) -> p two q", two=2)
                    for qc in range(2):
                        for ob in range(2):
                            nc.tensor.matmul(
                                psf[half * 4 + qc * 2 + ob][:],
                                aoc_r[:, :, qc * 128:(qc + 1) * 128],
                                wo_r[:, cp, :, ob * 512:(ob + 1) * 512],
                                start=(cp == 0), stop=(cp == 3), perf_mode=DR)
            for half in range(2):
                for qc in range(2):
                    for ob in range(2):
                        t = half * 4 + qc * 2 + ob
                        rb = half * 2 + qc
                        fo = p3s.tile([128, 512], FP, name=f"fo{t % 2}")
                        nc.vector.tensor_add(fo[:], psf[t][:],
                                             xb_s[rb][:, ob * 512:(ob + 1) * 512])
                        (nc.sync, nc.scalar, nc.gpsimd)[t % 3].dma_start(
                            out[rb * 128:(rb + 1) * 128,
                                ob * 512:(ob + 1) * 512], fo[:])

    if dbg is not None:
        for nm, t in (("d_qa", qa_t), ("d_qb", qb_t), ("d_k", kb_t),
                      ("d_vw", vw8)):
            nc.sync.dma_start(dbg[nm], t[:])
        nc.sync.dma_start(dbg["d_cc"][:, 0:NR // 2], cc_out[0][:])
        nc.sync.dma_start(dbg["d_cc"][:, NR // 2:NR], cc_out[1][:])
    dram.release()
    per.release()
    cst.release()


def build_nc(reps=1, use_collective=True, debug=False):
    nc = bacc.Bacc("TRN2", target_bir_lowering=False, debug=False, num_devices=NCORE)
    xt = nc.dram_tensor("xt", [D, N], F8, kind="ExternalInput").ap()
    xr = nc.dram_tensor("xr", [NR, D], FP, kind="ExternalInput").ap()
    wqt = nc.dram_tensor("wqt", [D, CB], F8, kind="ExternalInput").ap()
    wkt = nc.dram_tensor("wkt", [D, CB], F8, kind="ExternalInput").ap()
    wvt = nc.dram_tensor("wvt", [D, CB], F8, kind="ExternalInput").ap()
    wot = nc.dram_tensor("wot", [D, D], F8, kind="ExternalInput").ap()
    bqv = nc.dram_tensor("bqv", [CB], FP, kind="ExternalInput").ap()
    bkv = nc.dram_tensor("bkv", [CB], FP, kind="ExternalInput").ap()
    bvv = nc.dram_tensor("bvv", [CB], FP, kind="ExternalInput").ap()
    bov = nc.dram_tensor("bov", [D], FP, kind="ExternalInput").ap()
    # h pre-rearranged on host to [128, KCH]: hv[p, c] = h[c*128 + p]
    hv = nc.dram_tensor("hv", [128, KCH], FP, kind="ExternalInput").ap()
    out = nc.dram_tensor("out", [NR, D], FP, kind="ExternalOutput").ap()
    dbg = None
    if debug:
        dbg = {
            "d_qa": nc.dram_tensor("d_qa", [128, N], BF, kind="ExternalOutput").ap(),
            "d_qb": nc.dram_tensor("d_qb", [128, N], BF, kind="ExternalOutput").ap(),
            "d_k": nc.dram_tensor("d_k", [128, N], BF, kind="ExternalOutput").ap(),
            "d_vw": nc.dram_tensor("d_vw", [128, KCH * 256], F8, kind="ExternalOutput").ap(),
            "d_ao": nc.dram_tensor("d_ao", [128, N], F8, kind="ExternalOutput").ap(),
            "d_cc": nc.dram_tensor("d_cc", [NCORE * 128, NR], F8, kind="ExternalOutput").ap(),
            "d_sn0": nc.dram_tensor("d_sn0", [128, 512], FP, kind="ExternalOutput").ap(),
            "d_sn1": nc.dram_tensor("d_sn1", [128, 512], FP, kind="ExternalOutput").ap(),
        }
    with tile.TileContext(nc) as tc:
        _body(nc, tc, reps, xt, xr, wqt, wkt, wvt, wot,
              bqv, bkv, bvv, bov, hv, out, use_collective=use_collective, dbg=dbg)
    nc.compile()
    return nc


_NC_CACHE = {}


def get_nc(reps=1):
    if reps not in _NC_CACHE:
        _NC_CACHE[reps] = build_nc(reps)
    return _NC_CACHE[reps]


def make_in_maps(inputs):
    x = np.ascontiguousarray(np.asarray(inputs["x"], dtype=np.float32))
    h = np.ascontiguousarray(np.asarray(inputs["h"], dtype=np.float32))
    Wq = np.asarray(inputs["Wq"], dtype=np.float32)
    bq = np.asarray(inputs["bq"], dtype=np.float32)
    Wk = np.asarray(inputs["Wk"], dtype=np.float32)
    bk = np.asarray(inputs["bk"], dtype=np.float32)
    Wv = np.asarray(inputs["Wv"], dtype=np.float32)
    bv = np.asarray(inputs["bv"], dtype=np.float32)
    Wo = np.asarray(inputs["Wo"], dtype=np.float32)
    bo = np.ascontiguousarray(np.asarray(inputs["bo"], dtype=np.float32))
    import ml_dtypes
    bf16 = ml_dtypes.bfloat16
    f8 = ml_dtypes.float8_e4m3
    xt = np.ascontiguousarray(x.T.astype(f8))
    wot = np.ascontiguousarray(Wo.T.astype(f8))
    qscale = np.float32(0.125 * M8)  # 1/sqrt(dh) * Schraudolph scale
    in_maps = []
    for i in range(NCORE):
        cs = slice(i * CB, (i + 1) * CB)
        in_maps.append({
            "xt": xt,
            "xr": np.ascontiguousarray(x[i * NR:(i + 1) * NR, :]),
            "wqt": np.ascontiguousarray((Wq[cs, :] * qscale).T.astype(f8)),
            "wkt": np.ascontiguousarray(Wk[cs, :].T.astype(f8)),
            "wvt": np.ascontiguousarray(Wv[cs, :].T.astype(f8)),
            "wot": wot,
            "bqv": np.ascontiguousarray(bq[cs] * qscale),
            "bkv": np.ascontiguousarray(bk[cs]),
            "bvv": np.ascontiguousarray(bv[cs]),
            "bov": bo,
            "hv": np.ascontiguousarray(h.reshape(KCH, 128).T),
        })
    return in_maps


def kernel(**inputs):
    nc = get_nc(1)
    in_maps = make_in_maps(inputs)
    res = run_bass_kernel_spmd(nc, in_maps, core_ids=list(range(NCORE)))
    return np.concatenate([res.results[i]["out"] for i in range(NCORE)], axis=0)


# revision 37
# speedup vs baseline: 6.8032x; 1.0153x over previous
"""Trainium2 Bass kernel for nn_MILPAttention (dense multi-head attention with
per-key additive bias), tensor-parallel over heads across 8 NeuronCores.

Self-contained: hardcodes shapes N=4096, D=1024, H=16, GAMMA=1.0.

Math (reference):
    q = x @ Wq.T + bq ; k = x @ Wk.T + bk ; v = x @ Wv.T + bv     (per head, dh=64)
    logits = (q @ k.T) / 8 - h[key]
    attn = softmax(logits, keys)
    out = (attn @ v) @ Wo.T + bo + x

Per-core strategy (core i owns heads 2i, 2i+1 = columns 128i:128(i+1)):
  - Projections transposed: kT [128, 4096] = Wk.T.T @ x.T with x resident in
    SBUF (loaded once). q is computed into TWO half-zeroed tiles qA (head0 in
    rows 0:64, rows 64:128 = 0) and qB (head1 in rows 64:128, rows 0:64 = 0)
    so every S matmul runs with full K=128 contraction (K=64 matmuls run at
    half rate on TRN2). q is pre-scaled by m8 = 8/ln2 (Schraudolph prep).
  - S^T[key, q] in PSUM = m8 * logits. P' = exp(l - c) is produced in
    fp8e4m3, split between ScalarE (true exp via activation scale/bias) and
    VectorE (Schraudolph: int8 convert of m8*l + B' with saturating max,
    bits reinterpreted as fp8e4m3). The global shift c keeps P' in fp8
    range; it cancels in the softmax ratio.
  - P@V runs as fp8 DoubleRow matmuls (two key chunks per pass, 2x PE
    throughput). V is transposed via the PE, scaled by w = exp(-h) (folds
    the per-key bias), stored fp8 with w appended as a 65th column so the
    matmul also yields the softmax denominator.
  - Normalize uses reciprocal_approx_fast on the denominator rows, gpsimd
    partition-broadcast, and a vector multiply writing fp8. AllToAll (fp8)
    switches from head-sharding to sequence-sharding; the out projection
    runs as fp8 DoubleRow over chunk pairs, then bias + residual in fp32.
"""
import numpy as np

import concourse.bass as bass
import concourse.mybir as mybir
import concourse.tile as tile
from concourse import bacc
from concourse.bass_utils import run_bass_kernel_spmd
from concourse.masks import make_identity

N, D, H = 4096, 1024, 16
NCORE = 8
CB = D // NCORE          # 128 columns (2 heads) per core
NR = N // NCORE          # 512 output rows per core
DH = D // H              # 64
KCH = N // 128           # 32 key chunks
NB = N // 512            # 8 n-blocks
FP = mybir.dt.float32
BF = mybir.dt.bfloat16
F8 = mybir.dt.float8e4
I8 = mybir.dt.int8
AF = mybir.ActivationFunctionType
ALU = mybir.AluOpType
DR = mybir.MatmulPerfMode.DoubleRow

M8 = 8.0 / np.log(2.0)      # 11.5416 Schraudolph scale, folded into Wq on host
CSHIFT = 4.8                # global logit shift: P' = exp(l - c), cancels in softmax
                            # (fp8e4m3 here is IEEE e4m3: max normal 240, inf above)
CORR = -0.47                # Schraudolph correction (round-to-nearest tuned)
BPRIME = 56.0 + CORR - M8 * CSHIFT   # DVE add constant


def _body(nc, tc, reps, xt, xr, wqt, wkt, wvt, wot, bqv, bkv, bvv, bov, hv, out,
          use_collective=True, dbg=None):
    cst = tc.alloc_tile_pool(name="cst", bufs=1)
    per = tc.alloc_tile_pool(name="per", bufs=1)
    dram = tc.alloc_tile_pool(name="dram", bufs=1, space="DRAM")

    ident = cst.tile([128, 128], BF)
    make_identity(nc, ident[:])

    # persistent sbuf
    wq_b = per.tile([128, D], F8)        # [d-in-chunk, dc*128 + c]
    wk_b = per.tile([128, D], F8)
    wv_b = per.tile([128, D], F8)
    wo_b = per.tile([128, 8 * D], F8)    # [c-in-chunk, cc*1024 + o]
    qa_t = per.tile([128, N], BF)        # head0 q in rows 0:64, zeros 64:128
    qb_t = per.tile([128, N], BF)        # head1 q in rows 64:128, zeros 0:64
    kb_t = per.tile([128, N], BF)        # kT: rows = 2 heads x 64 dims
    vw8 = per.tile([128, KCH * 256], F8)  # per kc, per head: [w|pad63|v64]
    bq_s = per.tile([128, 1], FP)
    bk_s = per.tile([128, 1], FP)
    bv_s = per.tile([128, 1], FP)
    w_s = per.tile([128, KCH], FP)       # exp(-h), [key-in-chunk, chunk]
    cbias = per.tile([128, 1], FP)       # -CSHIFT for the exp activation
    cscale = per.tile([128, 1], FP)      # 1/M8 for the exp activation
    ones_r = per.tile([1, 128], FP)      # K=1 stationary for denom fan-out
    xb_s = [per.tile([128, D], FP, name=f"xb{j}") for j in range(4)]  # x rows + bo
    x_all = per.tile([128, 8 * N], F8)   # x^T resident, col = dc*N + n

    cc_in = [dram.tile([NCORE * 128, NR // 2], F8, name=f"ccin{i}") for i in range(2)]
    cc_out = [dram.tile([NCORE * 128, NR // 2], F8, name=f"ccout{i}") for i in range(2)]

    for rep in range(reps):
        sfx = f"_{rep}"
        # ---------------- phase 0: constants ------------------------------
        hst = per.tile([128, KCH], FP, name="hst")
        nc.sync.dma_start(hst[:], hv)
        nc.scalar.activation(w_s[:], hst[:], AF.Exp, scale=-1.0)
        nc.sync.dma_start(bq_s[:], bqv.unsqueeze(1))
        nc.scalar.dma_start(bk_s[:], bkv.unsqueeze(1))
        nc.scalar.dma_start(bv_s[:], bvv.unsqueeze(1))
        for wi, (wsrc, wdst) in enumerate(
                ((wqt, wq_b), (wkt, wk_b), (wvt, wv_b))):
            eng = (nc.sync, nc.scalar, nc.scalar)[wi]
            # [D, CB] bf16 -> [128, dc*128 + c] in one rearranged DMA
            eng.dma_start(wdst[:].rearrange("p (dc c) -> p dc c", c=CB),
                          wsrc.rearrange("(dc p) c -> p dc c", p=128))
        # zero halves of qA/qB (once per rep is harmless; writes are cheap)
        nc.vector.memset(qa_t[64:128, :], 0.0)
        nc.gpsimd.memset(qb_t[0:64, :], 0.0)
        nc.gpsimd.memset(cbias[:], -CSHIFT)
        nc.gpsimd.memset(cscale[:], 1.0 / M8)
        nc.gpsimd.memset(ones_r[:], 1.0)
        # w columns of vw8 (first col of each head block) <- w_s[:, kc]
        for h in range(2):
            nc.vector.tensor_copy(
                vw8[:].rearrange("p (kc c) -> p kc c", c=256)[:, :, 128 * h],
                w_s[:])

        # ------- phases 1+2 fully woven ------------------------------------
        # Projection blocks are emitted INSIDE the attention stream: k/v
        # blocks pipelined ahead of their first use, q blocks woven into
        # early q5 iterations. x^T is DMA'd once into resident SBUF tiles.
        with tc.tile_pool(name="p1s" + sfx, bufs=4) as p1s, \
             tc.tile_pool(name="aop" + sfx, bufs=4) as aop, \
             tc.tile_pool(name="p2s" + sfx, bufs=6) as p2s, \
             tc.tile_pool(name="p2n", bufs=2) as p2n, \
             tc.tile_pool(name="p2ps", bufs=2, space="PSUM") as p2ps, \
             tc.tile_pool(name="p2pb", bufs=2, space="PSUM") as p2pb, \
             tc.tile_pool(name="p2po", bufs=1, space="PSUM") as p2po:
            p1p = p2ps  # proj weave shares the psa buffers (name 'pss')
            def load_x(nb):
                for dc in range(8):
                    (nc.sync if dc % 2 == 0 else nc.gpsimd).dma_start(
                        x_all[:, dc * N + nb * 512:dc * N + (nb + 1) * 512],
                        xt[dc * 128:(dc + 1) * 128, nb * 512:(nb + 1) * 512])
                return None

            def kv_block(nb):
                ncol = slice(nb * 512, (nb + 1) * 512)
                psk = p1p.tile([128, 512], FP, name="pss")
                for dp in range(4):
                    nc.tensor.matmul(
                        psk[:],
                        wk_b[:].rearrange("p (dp two c) -> p dp two c",
                                          two=2, c=CB)[:, dp],
                        x_all[:].rearrange("p (dp two n) -> p dp two n",
                                           two=2, n=N)[:, dp, :, ncol],
                        start=(dp == 0), stop=(dp == 3), perf_mode=DR)
                nc.scalar.activation(kb_t[:, ncol], psk[:], AF.Identity, bias=bk_s[:, 0:1])
                psv = p1p.tile([128, 512], FP, name="pss")
                for dp in range(4):
                    nc.tensor.matmul(
                        psv[:],
                        wv_b[:].rearrange("p (dp two c) -> p dp two c",
                                          two=2, c=CB)[:, dp],
                        x_all[:].rearrange("p (dp two n) -> p dp two n",
                                           two=2, n=N)[:, dp, :, ncol],
                        start=(dp == 0), stop=(dp == 3), perf_mode=DR)
                vtb = p1s.tile([128, 512], BF, name="vtb")
                nc.scalar.activation(vtb[:], psv[:], AF.Identity, bias=bv_s[:, 0:1])
                for ns in range(4):
                    kc = nb * 4 + ns
                    pvt = p1p.tile([128, 128], BF, name="pss")
                    nc.tensor.transpose(pvt[:], vtb[:, ns * 128:(ns + 1) * 128], ident[:])
                    c0 = kc * 256
                    nc.vector.tensor_scalar_mul(
                        vw8[:, c0 + 64:c0 + 128], pvt[:, 0:64], w_s[:, kc:kc + 1])
                    nc.vector.tensor_scalar_mul(
                        vw8[:, c0 + 192:c0 + 256], pvt[:, 64:128], w_s[:, kc:kc + 1])

            def qproj_block(nb):
                ncol = slice(nb * 512, (nb + 1) * 512)
                psq = p1p.tile([128, 512], FP, name="pss")
                for dp in range(4):
                    nc.tensor.matmul(
                        psq[:],
                        wq_b[:].rearrange("p (dp two c) -> p dp two c",
                                          two=2, c=CB)[:, dp],
                        x_all[:].rearrange("p (dp two n) -> p dp two n",
                                           two=2, n=N)[:, dp, :, ncol],
                        start=(dp == 0), stop=(dp == 3), perf_mode=DR)
                qa_dst = qa_t[0:64, :].rearrange(
                    "p (s j t) -> p s j t", j=8, t=64)[:, :, nb, :]
                qb_dst = qb_t[64:128, :].rearrange(
                    "p (s j t) -> p s j t", j=8, t=64)[:, :, nb, :]
                nc.vector.tensor_scalar_add(
                    qa_dst, psq[0:64, :].rearrange("p (s t) -> p s t", t=64),
                    bq_s[0:64, 0:1])
                nc.vector.tensor_scalar_add(
                    qb_dst, psq[64:128, :].rearrange("p (s t) -> p s t", t=64),
                    bq_s[64:128, 0:1])

            # striped q layout: every phys q-block needs EVERY qproj block,
            # so x is fully preloaded and all q projections run up front.
            for nb in range(NB):
                load_x(nb)
            kv_block(0)
            for nb in range(NB):
                qproj_block(nb)
            kv_next = 1

            # vw8 stationary AP for (pair p, head h): [128, 2@130, 65]
            vw8_r = vw8[:].rearrange("p (pr two c) -> p pr two c", two=2, c=256)

            for q5 in range(N // 512):
                qcol = slice(q5 * 512, (q5 + 1) * 512)
                pso = [p2po.tile([128, 512], FP, name=f"pso{h}") for h in range(2)]
                pb_q = []  # deferred PV inputs: (pr, pb8_r)

                def emit_pv():
                    pr_, pbr_ = pb_q.pop(0)
                    for h in range(2):
                        nc.tensor.matmul(
                            pso[h][:], vw8_r[:, pr_, :, 128 * h:128 * h + 128],
                            pbr_[:, :, h * 512:(h + 1) * 512],
                            start=(pr_ == 0), stop=(pr_ == 15), perf_mode=DR)

                for pr in range(16):
                    pb8 = p2s.tile([128, 2048], F8, name="pb8")
                    pb8_r = pb8[:].rearrange("p (two c) -> p two c", two=2)
                    psb = p2pb.tile([128, 1024], FP, name="psb")
                    for i in range(2):
                        kc = 2 * pr + i
                        krng = slice(kc * 128, (kc + 1) * 128)
                        # head0 on ScalarE (true exp), head1 on VectorE
                        # (Schraudolph) — any per-engine bias is uniform
                        # within a head and cancels in the softmax ratio.
                        psa = p2ps.tile([128, 512], FP, name="pss")
                        nc.tensor.matmul(psa[:], kb_t[:, krng],
                                         qa_t[:, qcol], start=True, stop=True)
                        nc.scalar.activation(
                            pb8[:, i * 1024:i * 1024 + 512], psa[:],
                            AF.Exp, bias=cbias[:], scale=cscale[:])
                        nc.tensor.matmul(psb[:, i * 512:(i + 1) * 512],
                                         kb_t[:, krng],
                                         qb_t[:, qcol], start=True, stop=True)
                    # one paired Schraudolph op for both head1 halves
                    nc.vector.tensor_scalar(
                        pb8[:].bitcast(I8).rearrange(
                            "p (i c) -> p i c", c=1024)[:, :, 512:1024],
                        psb[:], BPRIME, 0.0, op0=ALU.add, op1=ALU.max)
                    pb_q.append((pr, pb8_r))
                    if pr > 0:
                        emit_pv()  # pair pr-1: its exps finished during this pair
                    # weave k/v projection blocks under the attention stream
                    if q5 == 0 and pr % 2 == 0 and kv_next < NB:
                        kv_block(kv_next)
                        kv_next += 1
                emit_pv()  # last pair
                if q5 == 4 and use_collective:
                    # first-half A2A runs under the remaining attention
                    nc.gpsimd.collective_compute(
                        "AllToAll", mybir.AluOpType.bypass,
                        replica_groups=[list(range(NCORE))],
                        ins=[cc_in[0][:].opt()], outs=[cc_out[0][:].opt()])
                # normalize: ao = snap[0:64] * recip(snap[64]) -> fp8
                # (all DVE ops keep matching partition offsets; row placement
                # into the A2A buffer is done by the staging DMAs)
                for h in range(2):
                    snap = p2n.tile([128, 512], FP, name=f"snap{h}")
                    nc.scalar.copy(snap[:], pso[h][:])
                    rc = p2n.tile([1, 512], FP, name=f"rc{h}")
                    nc.vector.reciprocal_approx_fast(rc[:], snap[0:1, :])
                    if dbg is not None and q5 == 0:
                        nc.sync.dma_start(dbg[f"d_sn{h}"], snap[:])
                    bch = p2n.tile([128, 512], FP, name=f"bc{h}")
                    nc.gpsimd.partition_broadcast(bch[:], rc[:])
                    # rotating output tile: avoids a tile-level WAR between
                    # this write and the previous block's staging-DMA read
                    aot = aop.tile([128, 512], F8, name=f"ao{h}")
                    # mul on VectorE: keeps the gpsimd queue free to issue the
                    # next broadcast immediately, overlapping the ~15us
                    # completion-semaphore latency of each broadcast
                    nc.vector.tensor_mul(aot[64:128, :], snap[64:128, :],
                                         bch[64:128, :])
                    # stage phys block q5 (stripe q5 of every dest) for the A2A
                    half, bc = q5 // 4, q5 % 4
                    nc.sync.dma_start(
                        cc_in[half][:].rearrange("(j g) c -> g j c", g=128)
                        [h * 64:(h + 1) * 64, :, bc * 64:(bc + 1) * 64],
                        aot[64:128, :].rearrange("p (j t) -> p j t", t=64))
                    if dbg is not None:
                        nc.scalar.dma_start(
                            dbg["d_ao"][h * 64:(h + 1) * 64, qcol],
                            aot[64:128, :])

        # prefetch phase-3 constants during phase 2 (DMA idle there)
        with tc.tile_pool(name="pf" + sfx, bufs=2) as pf:
            nc.sync.dma_start(wo_b[:].rearrange("p (cc o) -> p cc o", o=D),
                              wot.rearrange("(cc p) o -> p cc o", p=128))
            bost = pf.tile([128, D], FP, name="bost")
            nc.sync.dma_start(bost[:], bov.unsqueeze(0).broadcast_to([128, D]))
            for j in range(4):
                xrt = pf.tile([128, D], FP, name="xrt")
                (nc.sync if j % 2 == 0 else nc.scalar).dma_start(
                    xrt[:], xr[j * 128:(j + 1) * 128, :])
                nc.vector.tensor_add(xb_s[j][:], xrt[:], bost[:])

        # ---------------- phase 3: A2A + out projection (fp8 DoubleRow) ----
        with tc.tile_pool(name="p3s" + sfx, bufs=8) as p3s, \
             tc.tile_pool(name="p3p", bufs=1, space="PSUM") as p3p:
            if use_collective:
                nc.gpsimd.collective_compute(
                    "AllToAll", mybir.AluOpType.bypass,
                    replica_groups=[list(range(NCORE))],
                    ins=[cc_in[1][:].opt()], outs=[cc_out[1][:].opt()])
            else:  # single-core timing-sim stand-in
                for i in range(2):
                    nc.sync.dma_start(cc_out[i][:], cc_in[i][:])
            psf = [p3p.tile([128, 512], FP, name=f"psf{t}") for t in range(8)]
            wo_r = wo_b[:].rearrange("p (pr two o) -> p pr two o", two=2, o=D)
            for half in range(2):
                # issue ALL aoc loads for this half up front (bufs=8): their
                # ~15us DMA completion semaphores overlap instead of chaining
                aocs = []
                for cp in range(4):
                    aoc = p3s.tile([128, 512], F8, name="aoc")
                    (nc.sync, nc.scalar, nc.gpsimd, nc.sync)[cp].dma_start(
                        aoc[:].rearrange("p (two q) -> p two q", two=2),
                        cc_out[half][cp * 256:(cp + 1) * 256, :].rearrange(
                            "(two p) q -> p two q", p=128))
                    aocs.append(aoc)
                for cp in range(4):
                    aoc_r = aocs[cp][:].rearrange("p (two q) -> p two q", two=2)
                    for qc in range(2):
                        for ob in range(2):
                            nc.tensor.matmul(
                                psf[half * 4 + qc * 2 + ob][:],
                                aoc_r[:, :, qc * 128:(qc + 1) * 128],
                                wo_r[:, cp, :, ob * 512:(ob + 1) * 512],
                                start=(cp == 0), stop=(cp == 3), perf_mode=DR)
            for half in range(2):
                for qc in range(2):
                    for ob in range(2):
                        t = half * 4 + qc * 2 + ob
                        rb = half * 2 + qc
                        fo = p3s.tile([128, 512], FP, name=f"fo{t % 2}")
                        nc.vector.tensor_add(fo[:], psf[t][:],
                                             xb_s[rb][:, ob * 512:(ob + 1) * 512])
                        (nc.sync, nc.scalar, nc.gpsimd)[t % 3].dma_start(
                            out[rb * 128:(rb + 1) * 128,
                                ob * 512:(ob + 1) * 512], fo[:])

    if dbg is not None:
        for nm, t in (("d_qa", qa_t), ("d_qb", qb_t), ("d_k", kb_t),
                      ("d_vw", vw8)):
            nc.sync.dma_start(dbg[nm], t[:])
        nc.sync.dma_start(dbg["d_cc"][:, 0:NR // 2], cc_out[0][:])
        nc.sync.dma_start(dbg["d_cc"][:, NR // 2:NR], cc_out[1][:])
    dram.release()
    per.release()
    cst.release()


def build_nc(reps=1, use_collective=True, debug=False):
    nc = bacc.Bacc("TRN2", target_bir_lowering=False, debug=False, num_devices=NCORE)
    xt = nc.dram_tensor("xt", [D, N], F8, kind="ExternalInput").ap()
    xr = nc.dram_tensor("xr", [NR, D], FP, kind="ExternalInput").ap()
    wqt = nc.dram_tensor("wqt", [D, CB], F8, kind="ExternalInput").ap()
    wkt = nc.dram_tensor("wkt", [D, CB], F8, kind="ExternalInput").ap()
    wvt = nc.dram_tensor("wvt", [D, CB], F8, kind="ExternalInput").ap()
    wot = nc.dram_tensor("wot", [D, D], F8, kind="ExternalInput").ap()
    bqv = nc.dram_tensor("bqv", [CB], FP, kind="ExternalInput").ap()
    bkv = nc.dram_tensor("bkv", [CB], FP, kind="ExternalInput").ap()
    bvv = nc.dram_tensor("bvv", [CB], FP, kind="ExternalInput").ap()
    bov = nc.dram_tensor("bov", [D], FP, kind="ExternalInput").ap()
    # h pre-rearranged on host to [128, KCH]: hv[p, c] = h[c*128 + p]
    hv = nc.dram_tensor("hv", [128, KCH], FP, kind="ExternalInput").ap()
    out = nc.dram_tensor("out", [NR, D], FP, kind="ExternalOutput").ap()
    dbg = None
    if debug:
        dbg = {
            "d_qa": nc.dram_tensor("d_qa", [128, N], BF, kind="ExternalOutput").ap(),
            "d_qb": nc.dram_tensor("d_qb", [128, N], BF, kind="ExternalOutput").ap(),
            "d_k": nc.dram_tensor("d_k", [128, N], BF, kind="ExternalOutput").ap(),
            "d_vw": nc.dram_tensor("d_vw", [128, KCH * 256], F8, kind="ExternalOutput").ap(),
            "d_ao": nc.dram_tensor("d_ao", [128, N], F8, kind="ExternalOutput").ap(),
            "d_cc": nc.dram_tensor("d_cc", [NCORE * 128, NR], F8, kind="ExternalOutput").ap(),
            "d_sn0": nc.dram_tensor("d_sn0", [128, 512], FP, kind="ExternalOutput").ap(),
            "d_sn1": nc.dram_tensor("d_sn1", [128, 512], FP, kind="ExternalOutput").ap(),
        }
    with tile.TileContext(nc) as tc:
        _body(nc, tc, reps, xt, xr, wqt, wkt, wvt, wot,
              bqv, bkv, bvv, bov, hv, out, use_collective=use_collective, dbg=dbg)
    nc.compile()
    return nc


_NC_CACHE = {}


def get_nc(reps=1):
    if reps not in _NC_CACHE:
        _NC_CACHE[reps] = build_nc(reps)
    return _NC_CACHE[reps]


def make_in_maps(inputs):
    x = np.ascontiguousarray(np.asarray(inputs["x"], dtype=np.float32))
    h = np.ascontiguousarray(np.asarray(inputs["h"], dtype=np.float32))
    Wq = np.asarray(inputs["Wq"], dtype=np.float32)
    bq = np.asarray(inputs["bq"], dtype=np.float32)
    Wk = np.asarray(inputs["Wk"], dtype=np.float32)
    bk = np.asarray(inputs["bk"], dtype=np.float32)
    Wv = np.asarray(inputs["Wv"], dtype=np.float32)
    bv = np.asarray(inputs["bv"], dtype=np.float32)
    Wo = np.asarray(inputs["Wo"], dtype=np.float32)
    bo = np.ascontiguousarray(np.asarray(inputs["bo"], dtype=np.float32))
    import ml_dtypes
    bf16 = ml_dtypes.bfloat16
    f8 = ml_dtypes.float8_e4m3
    xt = np.ascontiguousarray(x.T.astype(f8))
    wot = np.ascontiguousarray(Wo.T.astype(f8))
    qscale = np.float32(0.125 * M8)  # 1/sqrt(dh) * Schraudolph scale
    in_maps = []
    for i in range(NCORE):
        cs = slice(i * CB, (i + 1) * CB)
        in_maps.append({
            "xt": xt,
            "xr": np.ascontiguousarray(x[i * NR:(i + 1) * NR, :]),
            "wqt": np.ascontiguousarray((Wq[cs, :] * qscale).T.astype(f8)),
            "wkt": np.ascontiguousarray(Wk[cs, :].T.astype(f8)),
            "wvt": np.ascontiguousarray(Wv[cs, :].T.astype(f8)),
            "wot": wot,
            "bqv": np.ascontiguousarray(bq[cs] * qscale),
            "bkv": np.ascontiguousarray(bk[cs]),
            "bvv": np.ascontiguousarray(bv[cs]),
            "bov": bo,
            "hv": np.ascontiguousarray(h.reshape(KCH, 128).T),
        })
    return in_maps


def kernel(**inputs):
    nc = get_nc(1)
    in_maps = make_in_maps(inputs)
    res = run_bass_kernel_spmd(nc, in_maps, core_ids=list(range(NCORE)))
    return np.concatenate([res.results[i]["out"] for i in range(NCORE)], axis=0)
